# revision 12
# baseline (speedup 1.0000x reference)
import numpy as np
from contextlib import ExitStack

import ml_dtypes
import concourse.bass as bass
import concourse.bacc as bacc
import concourse.mybir as mybir
from concourse.tile import TileContext
from concourse.bass_utils import run_bass_kernel_spmd

ALPHA = 3.0
N = 8192
DIM = 64
CORES = 8
RPC = N // CORES
P = 128
TILES = RPC // P
MMW = 512
PSB = 2048
F32 = mybir.dt.float32
BF16 = mybir.dt.bfloat16
FP8 = mybir.dt.float8e4
NPF8 = ml_dtypes.float8_e4m3
NPBF16 = ml_dtypes.bfloat16

MARGIN = np.float32(0.234)
THETA_BIG = np.float32(192.0)

_prog_cache: dict = {}
_jnp_mod = None


def _jnp():
    global _jnp_mod
    if _jnp_mod is None:
        import jax.numpy as jnp
        _jnp_mod = jnp
    return _jnp_mod


def _build_program() -> bass.Bass:
    nc = bacc.Bacc("TRN2", target_bir_lowering=False, debug=False,
                   num_devices=CORES)
    wxa_d = nc.dram_tensor("wxa", [P, RPC + PSB], BF16, kind="ExternalInput").ap()
    wxb_d = nc.dram_tensor("wxb", [P, RPC + (N - PSB)], BF16,
                           kind="ExternalInput").ap()
    ni_d = nc.dram_tensor("negi", [P, P], FP8, kind="ExternalInput").ap()
    th_d = nc.dram_tensor("th", [RPC, N], FP8, kind="ExternalInput").ap()
    out_d = nc.dram_tensor("out", [RPC, N], FP8, kind="ExternalOutput").ap()

    with TileContext(nc) as tc, ExitStack() as ctx:
        const_pool = ctx.enter_context(tc.tile_pool(name="const", bufs=1))
        t_pool = ctx.enter_context(tc.tile_pool(name="tpool", bufs=3))
        o_pool = ctx.enter_context(tc.tile_pool(name="opool", bufs=2))
        ps_pool = ctx.enter_context(
            tc.tile_pool(name="psum", bufs=2, space="PSUM"))

        wxa_sb = const_pool.tile([P, RPC + PSB], BF16)
        nc.sync.dma_start(wxa_sb[:], wxa_d[:])
        ni_sb = const_pool.tile([P, P], FP8)
        nc.sync.dma_start(ni_sb[:], ni_d[:])
        T0 = t_pool.tile([P, N], FP8, tag="T")
        for q in range(N // PSB):
            nc.sync.dma_start(T0[:, q * PSB:(q + 1) * PSB],
                              th_d[0:P, q * PSB:(q + 1) * PSB])
        wxb_sb = const_pool.tile([P, RPC + (N - PSB)], BF16)
        nc.sync.dma_start(wxb_sb[:], wxb_d[:])

        ps_warm = ps_pool.tile([P, PSB], F32, tag="ps")
        for _ in range(20):
            nc.tensor.matmul(ps_warm[:, 0:MMW], wxa_sb[:, 0:P], wxa_sb[:, 0:MMW],
                             start=True, stop=True)

        for m in range(TILES):
            if m == 0:
                T = T0
            else:
                T = t_pool.tile([P, N], FP8, tag="T")
                nc.sync.dma_start(T[:], th_d[m * P:(m + 1) * P, :])

            O = o_pool.tile([P, N], FP8, tag="O")
            for nb in range(N // PSB):
                src = wxa_sb if nb == 0 else wxb_sb
                base = RPC if nb == 0 else RPC + (nb - 1) * PSB
                act_chunk = (nb % 2 == 0)
                ps = ps_pool.tile([P, PSB], F32, tag="ps")
                for h in range(PSB // MMW):
                    nc.tensor.matmul(ps[:, h * MMW:(h + 1) * MMW],
                                     src[:, m * P:(m + 1) * P],
                                     src[:, base + h * MMW:base + (h + 1) * MMW],
                                     start=True, stop=not act_chunk)
                if act_chunk:
                    for h in range(PSB // MMW):
                        nc.tensor.matmul(ps[:, h * MMW:(h + 1) * MMW],
                                         ni_sb[:],
                                         T[:, nb * PSB + h * MMW:nb * PSB + (h + 1) * MMW],
                                         start=False, stop=True)
                Oc = O[:, nb * PSB:(nb + 1) * PSB]
                if act_chunk:
                    nc.scalar.activation(Oc, ps[:],
                                         mybir.ActivationFunctionType.Identity,
                                         bias=0.0, scale=1.0)
                else:
                    nc.vector.scalar_tensor_tensor(
                        Oc, ps[:], 1.0, T[:, nb * PSB:(nb + 1) * PSB],
                        mybir.AluOpType.mult, mybir.AluOpType.subtract)
            nc.gpsimd.dma_start(out_d[m * P:(m + 1) * P, :], O[:])
    nc.finalize()
    return nc


def get_program() -> bass.Bass:
    if "p" not in _prog_cache:
        _prog_cache["p"] = _build_program()
    return _prog_cache["p"]


def _host_nv(idx, emb1, emb2, lin1_w, lin1_b, lin2_w, lin2_b):
    jnp = _jnp()
    idx = jnp.asarray(idx)
    nv1 = jnp.tanh(ALPHA * (jnp.asarray(emb1)[idx] @ jnp.asarray(lin1_w).T
                            + jnp.asarray(lin1_b)))
    nv2 = jnp.tanh(ALPHA * (jnp.asarray(emb2)[idx] @ jnp.asarray(lin2_w).T
                            + jnp.asarray(lin2_b)))
    return np.asarray(nv1), np.asarray(nv2)


def _tanh_ref(x_f32):
    jnp = _jnp()
    n = x_f32.shape[0]
    L = 1 << 20
    while L < n:
        L <<= 1
    buf = np.zeros(L, np.float32)
    buf[:n] = x_f32
    return np.asarray(jnp.tanh(jnp.asarray(buf)))[:n]


def _sparse_dot(X, W, rows, cols, chunk=1 << 19):
    out = np.empty(len(rows), np.float32)
    for i in range(0, len(rows), chunk):
        sl = slice(i, i + chunk)
        out[sl] = (X[rows[sl]] * W[cols[sl]]).sum(axis=1, dtype=np.float32)
    return out


def _round_down_fp8(x_f32):
    t8 = x_f32.astype(NPF8)
    t8f = t8.astype(np.float32)
    b = t8.view(np.uint8)
    up = t8f > x_f32
    neg = np.signbit(t8f)
    b = np.where(up & ~neg, b - 1, np.where(up & neg, b + 1, b))
    return b.astype(np.uint8).view(NPF8)


def _row_reference(X, W, noise_row, r, k):
    a = (W @ X[r]).astype(np.float32)
    tv = _tanh_ref(ALPHA * a)
    adj = np.maximum(tv, np.float32(0.0))
    s = (adj + noise_row * np.float32(0.01)).astype(np.float32)
    order = np.lexsort((np.arange(N), -s))[:k]
    row = np.zeros(N, np.float32)
    row[order] = adj[order]
    return row


def kernel(idx, emb1, emb2, lin1_w, lin1_b, lin2_w, lin2_b, noise, k,
           _trace=False):
    k = int(k)
    noise_f = np.ascontiguousarray(np.asarray(noise, dtype=np.float32))
    nv1, nv2 = _host_nv(idx, emb1, emb2, lin1_w, lin1_b, lin2_w, lin2_b)

    X = np.concatenate([nv1, -nv2], axis=1).astype(np.float32)
    W = np.concatenate([nv2, nv1], axis=1).astype(np.float32)
    XT = np.ascontiguousarray(X.T).astype(NPBF16)
    WT = np.ascontiguousarray(W.T).astype(NPBF16)

    R = min(N - 1, 8 * k)
    tau_u = np.partition(noise_f, N - R, axis=1)[:, N - R]
    tau = (np.float32(1.0) + np.float32(0.01) * tau_u).astype(np.float32)
    g = (np.float32(0.01) * noise_f
         - (tau[:, None] - np.float32(1.0))).astype(np.float32)
    LN2 = np.float32(np.log(2.0))
    with np.errstate(divide="ignore", invalid="ignore"):
        theta = np.where(
            g > 0,
            (LN2 - np.log(g)) * np.float32(1.0 / 6.0) - MARGIN,
            THETA_BIG).astype(np.float32)
    th8 = _round_down_fp8(theta)

    nc = get_program()
    negi = (-np.eye(P, dtype=np.float32)).astype(NPF8)
    in_maps = [{
        "wxa": np.ascontiguousarray(
            np.concatenate([XT[:, c * RPC:(c + 1) * RPC], WT[:, :PSB]], axis=1)),
        "wxb": np.ascontiguousarray(
            np.concatenate([XT[:, c * RPC:(c + 1) * RPC], WT[:, PSB:]], axis=1)),
        "negi": negi,
        "th": np.ascontiguousarray(th8[c * RPC:(c + 1) * RPC]),
    } for c in range(CORES)]

    res = run_bass_kernel_spmd(nc, in_maps, core_ids=list(range(CORES)),
                               trace=_trace)
    zb = np.concatenate(
        [res.results[c]["out"].view(np.uint8) for c in range(CORES)], axis=0)

    cand = zb < 0x80
    rows, cols = np.nonzero(cand)
    a_rc = _sparse_dot(X, W, rows, cols)
    tv = _tanh_ref(np.float32(ALPHA) * a_rc)
    adjv = np.maximum(tv, np.float32(0.0))
    sv = (adjv + noise_f[rows, cols] * np.float32(0.01)).astype(np.float32)

    order = np.lexsort((cols, -sv, rows))
    rs, cs = rows[order], cols[order]
    sso, avo = sv[order], adjv[order]
    counts = np.bincount(rs, minlength=N)
    row_starts = np.concatenate(([0], np.cumsum(counts)))[:-1]
    pos = np.arange(len(rs)) - row_starts[rs]
    keep = pos < k

    out = np.zeros((N, N), np.float32)
    out[rs[keep], cs[keep]] = avo[keep]

    cnt_tau = np.bincount(rs[sso >= tau[rs]], minlength=N)
    bad = np.flatnonzero((cnt_tau < k) | (counts < k))
    for r in bad:
        out[r] = _row_reference(X, W, noise_f[r], r, k)

    out[np.arange(N), np.arange(N)] += np.float32(1.0)
    if _trace:
        return out, res
    return out


# revision 14
# speedup vs baseline: 1.0642x; 1.0642x over previous
import numpy as np
from contextlib import ExitStack

import ml_dtypes
import concourse.bass as bass
import concourse.bacc as bacc
import concourse.mybir as mybir
from concourse.tile import TileContext
from concourse.bass_utils import run_bass_kernel_spmd

ALPHA = 3.0
N = 8192
DIM = 64
CORES = 8
RPC = N // CORES
P = 128
TILES = RPC // P
MMW = 512
PSB = 2048
F32 = mybir.dt.float32
BF16 = mybir.dt.bfloat16
FP8 = mybir.dt.float8e4
NPF8 = ml_dtypes.float8_e4m3
NPBF16 = ml_dtypes.bfloat16

MARGIN = np.float32(0.234)
THETA_BIG = np.float32(192.0)

_prog_cache: dict = {}
_jnp_mod = None


def _jnp():
    global _jnp_mod
    if _jnp_mod is None:
        import jax.numpy as jnp
        _jnp_mod = jnp
    return _jnp_mod


def _build_program() -> bass.Bass:
    nc = bacc.Bacc("TRN2", target_bir_lowering=False, debug=False,
                   num_devices=CORES)
    wxa_d = nc.dram_tensor("wxa", [P, RPC + PSB], BF16, kind="ExternalInput").ap()
    wxb_d = nc.dram_tensor("wxb", [P, RPC + (N - PSB)], BF16,
                           kind="ExternalInput").ap()
    ni_d = nc.dram_tensor("negi", [P, P], FP8, kind="ExternalInput").ap()
    th_d = nc.dram_tensor("th", [RPC, N], FP8, kind="ExternalInput").ap()
    out_d = nc.dram_tensor("out", [RPC, N], FP8, kind="ExternalOutput").ap()

    with TileContext(nc) as tc, ExitStack() as ctx:
        const_pool = ctx.enter_context(tc.tile_pool(name="const", bufs=1))
        t_pool = ctx.enter_context(tc.tile_pool(name="tpool", bufs=3))
        o_pool = ctx.enter_context(tc.tile_pool(name="opool", bufs=2))
        ps_pool = ctx.enter_context(
            tc.tile_pool(name="psum", bufs=2, space="PSUM"))

        wxa_sb = const_pool.tile([P, RPC + PSB], BF16)
        nc.sync.dma_start(wxa_sb[:], wxa_d[:])
        ni_sb = const_pool.tile([P, P], FP8)
        nc.sync.dma_start(ni_sb[:], ni_d[:])
        T0 = t_pool.tile([P, N], FP8, tag="T")
        for q in range(N // PSB):
            nc.sync.dma_start(T0[:, q * PSB:(q + 1) * PSB],
                              th_d[0:P, q * PSB:(q + 1) * PSB])
        wxb_sb = const_pool.tile([P, RPC + (N - PSB)], BF16)
        nc.sync.dma_start(wxb_sb[:], wxb_d[:])

        ps_warm = ps_pool.tile([P, PSB], F32, tag="ps")
        for _ in range(20):
            nc.tensor.matmul(ps_warm[:, 0:MMW], wxa_sb[:, 0:P], wxa_sb[:, 0:MMW],
                             start=True, stop=True)

        for m in range(TILES):
            if m == 0:
                T = T0
            else:
                T = t_pool.tile([P, N], FP8, tag="T")
                for q in range(2):
                    h2 = N // 2
                    nc.sync.dma_start(
                        T[:, q * h2:(q + 1) * h2],
                        th_d[m * P:(m + 1) * P, q * h2:(q + 1) * h2])

            O = o_pool.tile([P, N], FP8, tag="O")
            for nb in range(N // PSB):
                src = wxa_sb if nb == 0 else wxb_sb
                base = RPC if nb == 0 else RPC + (nb - 1) * PSB
                act_chunk = (nb % 2 == 0)
                ps = ps_pool.tile([P, PSB], F32, tag="ps")
                for h in range(PSB // MMW):
                    nc.tensor.matmul(ps[:, h * MMW:(h + 1) * MMW],
                                     src[:, m * P:(m + 1) * P],
                                     src[:, base + h * MMW:base + (h + 1) * MMW],
                                     start=True, stop=not act_chunk)
                if act_chunk:
                    for h in range(PSB // MMW):
                        nc.tensor.matmul(ps[:, h * MMW:(h + 1) * MMW],
                                         ni_sb[:],
                                         T[:, nb * PSB + h * MMW:nb * PSB + (h + 1) * MMW],
                                         start=False, stop=True)
                Oc = O[:, nb * PSB:(nb + 1) * PSB]
                if act_chunk:
                    nc.scalar.activation(Oc, ps[:],
                                         mybir.ActivationFunctionType.Identity,
                                         bias=0.0, scale=1.0)
                else:
                    nc.vector.scalar_tensor_tensor(
                        Oc, ps[:], 1.0, T[:, nb * PSB:(nb + 1) * PSB],
                        mybir.AluOpType.mult, mybir.AluOpType.subtract)
                if nb % 2 == 1:
                    nc.gpsimd.dma_start(
                        out_d[m * P:(m + 1) * P, (nb - 1) * PSB:(nb + 1) * PSB],
                        O[:, (nb - 1) * PSB:(nb + 1) * PSB])
    nc.finalize()
    return nc


def get_program() -> bass.Bass:
    if "p" not in _prog_cache:
        _prog_cache["p"] = _build_program()
    return _prog_cache["p"]


def _host_nv(idx, emb1, emb2, lin1_w, lin1_b, lin2_w, lin2_b):
    jnp = _jnp()
    idx = jnp.asarray(idx)
    nv1 = jnp.tanh(ALPHA * (jnp.asarray(emb1)[idx] @ jnp.asarray(lin1_w).T
                            + jnp.asarray(lin1_b)))
    nv2 = jnp.tanh(ALPHA * (jnp.asarray(emb2)[idx] @ jnp.asarray(lin2_w).T
                            + jnp.asarray(lin2_b)))
    return np.asarray(nv1), np.asarray(nv2)


def _tanh_ref(x_f32):
    jnp = _jnp()
    n = x_f32.shape[0]
    L = 1 << 20
    while L < n:
        L <<= 1
    buf = np.zeros(L, np.float32)
    buf[:n] = x_f32
    return np.asarray(jnp.tanh(jnp.asarray(buf)))[:n]


def _sparse_dot(X, W, rows, cols, chunk=1 << 19):
    out = np.empty(len(rows), np.float32)
    for i in range(0, len(rows), chunk):
        sl = slice(i, i + chunk)
        out[sl] = (X[rows[sl]] * W[cols[sl]]).sum(axis=1, dtype=np.float32)
    return out


def _round_down_fp8(x_f32):
    t8 = x_f32.astype(NPF8)
    t8f = t8.astype(np.float32)
    b = t8.view(np.uint8)
    up = t8f > x_f32
    neg = np.signbit(t8f)
    b = np.where(up & ~neg, b - 1, np.where(up & neg, b + 1, b))
    return b.astype(np.uint8).view(NPF8)


def _row_reference(X, W, noise_row, r, k):
    a = (W @ X[r]).astype(np.float32)
    tv = _tanh_ref(ALPHA * a)
    adj = np.maximum(tv, np.float32(0.0))
    s = (adj + noise_row * np.float32(0.01)).astype(np.float32)
    order = np.lexsort((np.arange(N), -s))[:k]
    row = np.zeros(N, np.float32)
    row[order] = adj[order]
    return row


def kernel(idx, emb1, emb2, lin1_w, lin1_b, lin2_w, lin2_b, noise, k,
           _trace=False):
    k = int(k)
    noise_f = np.ascontiguousarray(np.asarray(noise, dtype=np.float32))
    nv1, nv2 = _host_nv(idx, emb1, emb2, lin1_w, lin1_b, lin2_w, lin2_b)

    X = np.concatenate([nv1, -nv2], axis=1).astype(np.float32)
    W = np.concatenate([nv2, nv1], axis=1).astype(np.float32)
    XT = np.ascontiguousarray(X.T).astype(NPBF16)
    WT = np.ascontiguousarray(W.T).astype(NPBF16)

    R = min(N - 1, 8 * k)
    tau_u = np.partition(noise_f, N - R, axis=1)[:, N - R]
    tau = (np.float32(1.0) + np.float32(0.01) * tau_u).astype(np.float32)
    g = (np.float32(0.01) * noise_f
         - (tau[:, None] - np.float32(1.0))).astype(np.float32)
    LN2 = np.float32(np.log(2.0))
    with np.errstate(divide="ignore", invalid="ignore"):
        theta = np.where(
            g > 0,
            (LN2 - np.log(g)) * np.float32(1.0 / 6.0) - MARGIN,
            THETA_BIG).astype(np.float32)
    th8 = _round_down_fp8(theta)

    nc = get_program()
    negi = (-np.eye(P, dtype=np.float32)).astype(NPF8)
    in_maps = [{
        "wxa": np.ascontiguousarray(
            np.concatenate([XT[:, c * RPC:(c + 1) * RPC], WT[:, :PSB]], axis=1)),
        "wxb": np.ascontiguousarray(
            np.concatenate([XT[:, c * RPC:(c + 1) * RPC], WT[:, PSB:]], axis=1)),
        "negi": negi,
        "th": np.ascontiguousarray(th8[c * RPC:(c + 1) * RPC]),
    } for c in range(CORES)]

    res = run_bass_kernel_spmd(nc, in_maps, core_ids=list(range(CORES)),
                               trace=_trace)
    zb = np.concatenate(
        [res.results[c]["out"].view(np.uint8) for c in range(CORES)], axis=0)

    cand = zb < 0x80
    rows, cols = np.nonzero(cand)
    a_rc = _sparse_dot(X, W, rows, cols)
    tv = _tanh_ref(np.float32(ALPHA) * a_rc)
    adjv = np.maximum(tv, np.float32(0.0))
    sv = (adjv + noise_f[rows, cols] * np.float32(0.01)).astype(np.float32)

    order = np.lexsort((cols, -sv, rows))
    rs, cs = rows[order], cols[order]
    sso, avo = sv[order], adjv[order]
    counts = np.bincount(rs, minlength=N)
    row_starts = np.concatenate(([0], np.cumsum(counts)))[:-1]
    pos = np.arange(len(rs)) - row_starts[rs]
    keep = pos < k

    out = np.zeros((N, N), np.float32)
    out[rs[keep], cs[keep]] = avo[keep]

    cnt_tau = np.bincount(rs[sso >= tau[rs]], minlength=N)
    bad = np.flatnonzero((cnt_tau < k) | (counts < k))
    for r in bad:
        out[r] = _row_reference(X, W, noise_f[r], r, k)

    out[np.arange(N), np.arange(N)] += np.float32(1.0)
    if _trace:
        return out, res
    return out


# revision 15
# speedup vs baseline: 1.0967x; 1.0306x over previous
import numpy as np
from contextlib import ExitStack

import ml_dtypes
import concourse.bass as bass
import concourse.bacc as bacc
import concourse.mybir as mybir
from concourse.tile import TileContext
from concourse.bass_utils import run_bass_kernel_spmd

ALPHA = 3.0
N = 8192
DIM = 64
CORES = 8
RPC = N // CORES
P = 128
TILES = RPC // P
MMW = 512
PSB = 2048
F32 = mybir.dt.float32
BF16 = mybir.dt.bfloat16
FP8 = mybir.dt.float8e4
NPF8 = ml_dtypes.float8_e4m3
NPBF16 = ml_dtypes.bfloat16

MARGIN = np.float32(0.234)
THETA_BIG = np.float32(192.0)

_prog_cache: dict = {}
_jnp_mod = None


def _jnp():
    global _jnp_mod
    if _jnp_mod is None:
        import jax.numpy as jnp
        _jnp_mod = jnp
    return _jnp_mod


def _build_program() -> bass.Bass:
    nc = bacc.Bacc("TRN2", target_bir_lowering=False, debug=False,
                   num_devices=CORES)
    wxa_d = nc.dram_tensor("wxa", [P, RPC + PSB], BF16, kind="ExternalInput").ap()
    wxb_d = nc.dram_tensor("wxb", [P, RPC + (N - PSB)], BF16,
                           kind="ExternalInput").ap()
    ni_d = nc.dram_tensor("negi", [P, P], FP8, kind="ExternalInput").ap()
    th_d = nc.dram_tensor("th", [RPC, N], FP8, kind="ExternalInput").ap()
    out_d = nc.dram_tensor("out", [RPC, N], FP8, kind="ExternalOutput").ap()

    with TileContext(nc) as tc, ExitStack() as ctx:
        const_pool = ctx.enter_context(tc.tile_pool(name="const", bufs=1))
        t_pool = ctx.enter_context(tc.tile_pool(name="tpool", bufs=3))
        o_pool = ctx.enter_context(tc.tile_pool(name="opool", bufs=2))
        ps_pool = ctx.enter_context(
            tc.tile_pool(name="psum", bufs=2, space="PSUM"))

        wxa_sb = const_pool.tile([P, RPC + PSB], BF16)
        nc.sync.dma_start(wxa_sb[:], wxa_d[:])
        ni_sb = const_pool.tile([P, P], FP8)
        nc.sync.dma_start(ni_sb[:], ni_d[:])
        T0 = t_pool.tile([P, N], FP8, tag="T")
        for q in range(N // PSB):
            nc.gpsimd.dma_start(T0[:, q * PSB:(q + 1) * PSB],
                                th_d[0:P, q * PSB:(q + 1) * PSB])
        wxb_sb = const_pool.tile([P, RPC + (N - PSB)], BF16)
        nc.sync.dma_start(wxb_sb[:], wxb_d[:])

        ps_warm = ps_pool.tile([P, PSB], F32, tag="ps")
        for _ in range(20):
            nc.tensor.matmul(ps_warm[:, 0:MMW], wxa_sb[:, 0:P], wxa_sb[:, 0:MMW],
                             start=True, stop=True)

        for m in range(TILES):
            if m == 0:
                T = T0
            else:
                T = t_pool.tile([P, N], FP8, tag="T")
                for q in range(2):
                    h2 = N // 2
                    nc.gpsimd.dma_start(
                        T[:, q * h2:(q + 1) * h2],
                        th_d[m * P:(m + 1) * P, q * h2:(q + 1) * h2])

            O = o_pool.tile([P, N], FP8, tag="O")
            for nb in range(N // PSB):
                src = wxa_sb if nb == 0 else wxb_sb
                base = RPC if nb == 0 else RPC + (nb - 1) * PSB
                act_chunk = (nb % 2 == 0)
                ps = ps_pool.tile([P, PSB], F32, tag="ps")
                for h in range(PSB // MMW):
                    nc.tensor.matmul(ps[:, h * MMW:(h + 1) * MMW],
                                     src[:, m * P:(m + 1) * P],
                                     src[:, base + h * MMW:base + (h + 1) * MMW],
                                     start=True, stop=not act_chunk)
                if act_chunk:
                    for h in range(PSB // MMW):
                        nc.tensor.matmul(ps[:, h * MMW:(h + 1) * MMW],
                                         ni_sb[:],
                                         T[:, nb * PSB + h * MMW:nb * PSB + (h + 1) * MMW],
                                         start=False, stop=True)
                Oc = O[:, nb * PSB:(nb + 1) * PSB]
                if act_chunk:
                    nc.scalar.activation(Oc, ps[:],
                                         mybir.ActivationFunctionType.Identity,
                                         bias=0.0, scale=1.0)
                else:
                    nc.vector.scalar_tensor_tensor(
                        Oc, ps[:], 1.0, T[:, nb * PSB:(nb + 1) * PSB],
                        mybir.AluOpType.mult, mybir.AluOpType.subtract)
                if nb % 2 == 1:
                    nc.sync.dma_start(
                        out_d[m * P:(m + 1) * P, (nb - 1) * PSB:(nb + 1) * PSB],
                        O[:, (nb - 1) * PSB:(nb + 1) * PSB])
    nc.finalize()
    return nc


def get_program() -> bass.Bass:
    if "p" not in _prog_cache:
        _prog_cache["p"] = _build_program()
    return _prog_cache["p"]


def _host_nv(idx, emb1, emb2, lin1_w, lin1_b, lin2_w, lin2_b):
    jnp = _jnp()
    idx = jnp.asarray(idx)
    nv1 = jnp.tanh(ALPHA * (jnp.asarray(emb1)[idx] @ jnp.asarray(lin1_w).T
                            + jnp.asarray(lin1_b)))
    nv2 = jnp.tanh(ALPHA * (jnp.asarray(emb2)[idx] @ jnp.asarray(lin2_w).T
                            + jnp.asarray(lin2_b)))
    return np.asarray(nv1), np.asarray(nv2)


def _tanh_ref(x_f32):
    jnp = _jnp()
    n = x_f32.shape[0]
    L = 1 << 20
    while L < n:
        L <<= 1
    buf = np.zeros(L, np.float32)
    buf[:n] = x_f32
    return np.asarray(jnp.tanh(jnp.asarray(buf)))[:n]


def _sparse_dot(X, W, rows, cols, chunk=1 << 19):
    out = np.empty(len(rows), np.float32)
    for i in range(0, len(rows), chunk):
        sl = slice(i, i + chunk)
        out[sl] = (X[rows[sl]] * W[cols[sl]]).sum(axis=1, dtype=np.float32)
    return out


def _round_down_fp8(x_f32):
    t8 = x_f32.astype(NPF8)
    t8f = t8.astype(np.float32)
    b = t8.view(np.uint8)
    up = t8f > x_f32
    neg = np.signbit(t8f)
    b = np.where(up & ~neg, b - 1, np.where(up & neg, b + 1, b))
    return b.astype(np.uint8).view(NPF8)


def _row_reference(X, W, noise_row, r, k):
    a = (W @ X[r]).astype(np.float32)
    tv = _tanh_ref(ALPHA * a)
    adj = np.maximum(tv, np.float32(0.0))
    s = (adj + noise_row * np.float32(0.01)).astype(np.float32)
    order = np.lexsort((np.arange(N), -s))[:k]
    row = np.zeros(N, np.float32)
    row[order] = adj[order]
    return row


def kernel(idx, emb1, emb2, lin1_w, lin1_b, lin2_w, lin2_b, noise, k,
           _trace=False):
    k = int(k)
    noise_f = np.ascontiguousarray(np.asarray(noise, dtype=np.float32))
    nv1, nv2 = _host_nv(idx, emb1, emb2, lin1_w, lin1_b, lin2_w, lin2_b)

    X = np.concatenate([nv1, -nv2], axis=1).astype(np.float32)
    W = np.concatenate([nv2, nv1], axis=1).astype(np.float32)
    XT = np.ascontiguousarray(X.T).astype(NPBF16)
    WT = np.ascontiguousarray(W.T).astype(NPBF16)

    R = min(N - 1, 8 * k)
    tau_u = np.partition(noise_f, N - R, axis=1)[:, N - R]
    tau = (np.float32(1.0) + np.float32(0.01) * tau_u).astype(np.float32)
    g = (np.float32(0.01) * noise_f
         - (tau[:, None] - np.float32(1.0))).astype(np.float32)
    LN2 = np.float32(np.log(2.0))
    with np.errstate(divide="ignore", invalid="ignore"):
        theta = np.where(
            g > 0,
            (LN2 - np.log(g)) * np.float32(1.0 / 6.0) - MARGIN,
            THETA_BIG).astype(np.float32)
    th8 = _round_down_fp8(theta)

    nc = get_program()
    negi = (-np.eye(P, dtype=np.float32)).astype(NPF8)
    in_maps = [{
        "wxa": np.ascontiguousarray(
            np.concatenate([XT[:, c * RPC:(c + 1) * RPC], WT[:, :PSB]], axis=1)),
        "wxb": np.ascontiguousarray(
            np.concatenate([XT[:, c * RPC:(c + 1) * RPC], WT[:, PSB:]], axis=1)),
        "negi": negi,
        "th": np.ascontiguousarray(th8[c * RPC:(c + 1) * RPC]),
    } for c in range(CORES)]

    res = run_bass_kernel_spmd(nc, in_maps, core_ids=list(range(CORES)),
                               trace=_trace)
    zb = np.concatenate(
        [res.results[c]["out"].view(np.uint8) for c in range(CORES)], axis=0)

    cand = zb < 0x80
    rows, cols = np.nonzero(cand)
    a_rc = _sparse_dot(X, W, rows, cols)
    tv = _tanh_ref(np.float32(ALPHA) * a_rc)
    adjv = np.maximum(tv, np.float32(0.0))
    sv = (adjv + noise_f[rows, cols] * np.float32(0.01)).astype(np.float32)

    order = np.lexsort((cols, -sv, rows))
    rs, cs = rows[order], cols[order]
    sso, avo = sv[order], adjv[order]
    counts = np.bincount(rs, minlength=N)
    row_starts = np.concatenate(([0], np.cumsum(counts)))[:-1]
    pos = np.arange(len(rs)) - row_starts[rs]
    keep = pos < k

    out = np.zeros((N, N), np.float32)
    out[rs[keep], cs[keep]] = avo[keep]

    cnt_tau = np.bincount(rs[sso >= tau[rs]], minlength=N)
    bad = np.flatnonzero((cnt_tau < k) | (counts < k))
    for r in bad:
        out[r] = _row_reference(X, W, noise_f[r], r, k)

    out[np.arange(N), np.arange(N)] += np.float32(1.0)
    if _trace:
        return out, res
    return out


# revision 19
# speedup vs baseline: 1.1565x; 1.0545x over previous
import numpy as np
from contextlib import ExitStack

import ml_dtypes
import concourse.bass as bass
import concourse.bacc as bacc
import concourse.mybir as mybir
from concourse.tile import TileContext
from concourse.bass_utils import run_bass_kernel_spmd

ALPHA = 3.0
N = 8192
DIM = 64
CORES = 8
RPC = N // CORES
P = 128
TILES = RPC // P
MMW = 512
PSB = 2048
F32 = mybir.dt.float32
BF16 = mybir.dt.bfloat16
FP8 = mybir.dt.float8e4
NPF8 = ml_dtypes.float8_e4m3
NPBF16 = ml_dtypes.bfloat16

MARGIN = np.float32(0.234)
THETA_BIG = np.float32(192.0)

_prog_cache: dict = {}
_jnp_mod = None


def _jnp():
    global _jnp_mod
    if _jnp_mod is None:
        import jax.numpy as jnp
        _jnp_mod = jnp
    return _jnp_mod


def _build_program() -> bass.Bass:
    nc = bacc.Bacc("TRN2", target_bir_lowering=False, debug=False,
                   num_devices=CORES)
    wxa_d = nc.dram_tensor("wxa", [P, RPC + PSB], BF16, kind="ExternalInput").ap()
    wxb_d = nc.dram_tensor("wxb", [P, RPC + (N - PSB)], BF16,
                           kind="ExternalInput").ap()
    ni_d = nc.dram_tensor("negi", [P, P], FP8, kind="ExternalInput").ap()
    th_d = nc.dram_tensor("th", [RPC, N], FP8, kind="ExternalInput").ap()
    out_d = nc.dram_tensor("out", [RPC, N], FP8, kind="ExternalOutput").ap()

    with TileContext(nc) as tc, ExitStack() as ctx:
        const_pool = ctx.enter_context(tc.tile_pool(name="const", bufs=1))
        t_pool = ctx.enter_context(tc.tile_pool(name="tpool", bufs=3))
        o_pool = ctx.enter_context(tc.tile_pool(name="opool", bufs=3))
        ps_pool = ctx.enter_context(
            tc.tile_pool(name="psum", bufs=2, space="PSUM"))

        wxa_sb = const_pool.tile([P, RPC + PSB], BF16)
        nc.sync.dma_start(wxa_sb[:], wxa_d[:])
        ni_sb = const_pool.tile([P, P], FP8)
        nc.sync.dma_start(ni_sb[:], ni_d[:])
        T_first = t_pool.tile([P, N], FP8, tag="T")
        Ts = [T_first]
        for q in range(N // PSB):
            nc.sync.dma_start(Ts[0][:, q * PSB:(q + 1) * PSB],
                              th_d[0:P, q * PSB:(q + 1) * PSB])
        wxb_sb = const_pool.tile([P, RPC + (N - PSB)], BF16)
        nc.sync.dma_start(wxb_sb[:], wxb_d[:])

        def fetch_theta(mm):
            T = t_pool.tile([P, N], FP8, tag="T")
            h2 = N // 2
            for q in range(2):
                nc.sync.dma_start(T[:, q * h2:(q + 1) * h2],
                                  th_d[mm * P:(mm + 1) * P, q * h2:(q + 1) * h2])
            return T
        Ts.append(fetch_theta(1))

        ps_warm = ps_pool.tile([P, PSB], F32, tag="ps")
        for _ in range(20):
            nc.tensor.matmul(ps_warm[:, 0:MMW], wxa_sb[:, 0:P], wxa_sb[:, 0:MMW],
                             start=True, stop=True)

        for m in range(TILES):
            if m + 2 < TILES:
                Ts.append(fetch_theta(m + 2))
            T = Ts[m]

            O = o_pool.tile([P, N], FP8, tag="O")
            for nb in range(N // PSB):
                src = wxa_sb if nb == 0 else wxb_sb
                base = RPC if nb == 0 else RPC + (nb - 1) * PSB
                act_chunk = (nb % 2 == 0)
                ps = ps_pool.tile([P, PSB], F32, tag="ps")
                for h in range(PSB // MMW):
                    nc.tensor.matmul(ps[:, h * MMW:(h + 1) * MMW],
                                     src[:, m * P:(m + 1) * P],
                                     src[:, base + h * MMW:base + (h + 1) * MMW],
                                     start=True, stop=not act_chunk)
                if act_chunk:
                    for h in range(PSB // MMW):
                        nc.tensor.matmul(ps[:, h * MMW:(h + 1) * MMW],
                                         ni_sb[:],
                                         T[:, nb * PSB + h * MMW:nb * PSB + (h + 1) * MMW],
                                         start=False, stop=True)
                Oc = O[:, nb * PSB:(nb + 1) * PSB]
                if act_chunk:
                    nc.scalar.activation(Oc, ps[:],
                                         mybir.ActivationFunctionType.Identity,
                                         bias=0.0, scale=1.0)
                else:
                    nc.vector.scalar_tensor_tensor(
                        Oc, ps[:], 1.0, T[:, nb * PSB:(nb + 1) * PSB],
                        mybir.AluOpType.mult, mybir.AluOpType.subtract)
                if nb % 2 == 1:
                    nc.sync.dma_start(
                        out_d[m * P:(m + 1) * P, (nb - 1) * PSB:(nb + 1) * PSB],
                        O[:, (nb - 1) * PSB:(nb + 1) * PSB])
    nc.finalize()
    return nc


def get_program() -> bass.Bass:
    if "p" not in _prog_cache:
        _prog_cache["p"] = _build_program()
    return _prog_cache["p"]


def _host_nv(idx, emb1, emb2, lin1_w, lin1_b, lin2_w, lin2_b):
    jnp = _jnp()
    idx = jnp.asarray(idx)
    nv1 = jnp.tanh(ALPHA * (jnp.asarray(emb1)[idx] @ jnp.asarray(lin1_w).T
                            + jnp.asarray(lin1_b)))
    nv2 = jnp.tanh(ALPHA * (jnp.asarray(emb2)[idx] @ jnp.asarray(lin2_w).T
                            + jnp.asarray(lin2_b)))
    return np.asarray(nv1), np.asarray(nv2)


def _tanh_ref(x_f32):
    jnp = _jnp()
    n = x_f32.shape[0]
    L = 1 << 20
    while L < n:
        L <<= 1
    buf = np.zeros(L, np.float32)
    buf[:n] = x_f32
    return np.asarray(jnp.tanh(jnp.asarray(buf)))[:n]


def _sparse_dot(X, W, rows, cols, chunk=1 << 19):
    out = np.empty(len(rows), np.float32)
    for i in range(0, len(rows), chunk):
        sl = slice(i, i + chunk)
        out[sl] = (X[rows[sl]] * W[cols[sl]]).sum(axis=1, dtype=np.float32)
    return out


def _round_down_fp8(x_f32):
    t8 = x_f32.astype(NPF8)
    t8f = t8.astype(np.float32)
    b = t8.view(np.uint8)
    up = t8f > x_f32
    neg = np.signbit(t8f)
    b = np.where(up & ~neg, b - 1, np.where(up & neg, b + 1, b))
    return b.astype(np.uint8).view(NPF8)


def _row_reference(X, W, noise_row, r, k):
    a = (W @ X[r]).astype(np.float32)
    tv = _tanh_ref(ALPHA * a)
    adj = np.maximum(tv, np.float32(0.0))
    s = (adj + noise_row * np.float32(0.01)).astype(np.float32)
    order = np.lexsort((np.arange(N), -s))[:k]
    row = np.zeros(N, np.float32)
    row[order] = adj[order]
    return row


def kernel(idx, emb1, emb2, lin1_w, lin1_b, lin2_w, lin2_b, noise, k,
           _trace=False):
    k = int(k)
    noise_f = np.ascontiguousarray(np.asarray(noise, dtype=np.float32))
    nv1, nv2 = _host_nv(idx, emb1, emb2, lin1_w, lin1_b, lin2_w, lin2_b)

    X = np.concatenate([nv1, -nv2], axis=1).astype(np.float32)
    W = np.concatenate([nv2, nv1], axis=1).astype(np.float32)
    XT = np.ascontiguousarray(X.T).astype(NPBF16)
    WT = np.ascontiguousarray(W.T).astype(NPBF16)

    R = min(N - 1, 8 * k)
    tau_u = np.partition(noise_f, N - R, axis=1)[:, N - R]
    tau = (np.float32(1.0) + np.float32(0.01) * tau_u).astype(np.float32)
    g = (np.float32(0.01) * noise_f
         - (tau[:, None] - np.float32(1.0))).astype(np.float32)
    LN2 = np.float32(np.log(2.0))
    with np.errstate(divide="ignore", invalid="ignore"):
        theta = np.where(
            g > 0,
            (LN2 - np.log(g)) * np.float32(1.0 / 6.0) - MARGIN,
            THETA_BIG).astype(np.float32)
    th8 = _round_down_fp8(theta)

    nc = get_program()
    negi = (-np.eye(P, dtype=np.float32)).astype(NPF8)
    in_maps = [{
        "wxa": np.ascontiguousarray(
            np.concatenate([XT[:, c * RPC:(c + 1) * RPC], WT[:, :PSB]], axis=1)),
        "wxb": np.ascontiguousarray(
            np.concatenate([XT[:, c * RPC:(c + 1) * RPC], WT[:, PSB:]], axis=1)),
        "negi": negi,
        "th": np.ascontiguousarray(th8[c * RPC:(c + 1) * RPC]),
    } for c in range(CORES)]

    res = run_bass_kernel_spmd(nc, in_maps, core_ids=list(range(CORES)),
                               trace=_trace)
    zb = np.concatenate(
        [res.results[c]["out"].view(np.uint8) for c in range(CORES)], axis=0)

    cand = zb < 0x80
    rows, cols = np.nonzero(cand)
    a_rc = _sparse_dot(X, W, rows, cols)
    tv = _tanh_ref(np.float32(ALPHA) * a_rc)
    adjv = np.maximum(tv, np.float32(0.0))
    sv = (adjv + noise_f[rows, cols] * np.float32(0.01)).astype(np.float32)

    order = np.lexsort((cols, -sv, rows))
    rs, cs = rows[order], cols[order]
    sso, avo = sv[order], adjv[order]
    counts = np.bincount(rs, minlength=N)
    row_starts = np.concatenate(([0], np.cumsum(counts)))[:-1]
    pos = np.arange(len(rs)) - row_starts[rs]
    keep = pos < k

    out = np.zeros((N, N), np.float32)
    out[rs[keep], cs[keep]] = avo[keep]

    cnt_tau = np.bincount(rs[sso >= tau[rs]], minlength=N)
    bad = np.flatnonzero((cnt_tau < k) | (counts < k))
    for r in bad:
        out[r] = _row_reference(X, W, noise_f[r], r, k)

    out[np.arange(N), np.arange(N)] += np.float32(1.0)
    if _trace:
        return out, res
    return out
